# revision 15
# baseline (speedup 1.0000x reference)
"""MultiHeadLatentAttention Trainium2 kernel (8 NeuronCores, SPMD).

Sharding: batch (2) x head-group (4 of 4 heads each) -> 8 cores.
Each core computes, for its batch b and heads [4g, 4g+4):
  fused phase 1: latT = Wkv^T x^T + bkv AND QT = Wq^T x^T + bq from one
    xT stream (xT DMA'd once, 8 PSUM banks: 4 lat + 4 q per seq chunk;
    lat readout on ACT with fused bias, q readout on DVE)
  phase 2: KT = Wk_sl^T latT + bk (ACT readout+bias);
    V = latT^T Wv_sl + bv (DVE fused copy+bias, written bf16)
  phase 3 per head h, q-chunk j (512 wide), causal k-tiles paired:
    ST[k,q] = KT_h[:,ktile]^T QT_h[:,j]   (diagonal tiles: only cols
      >= 128*r computed; [128,128] triangle bias added on the diagonal)
    ET = exp(scale*ST) via ONE ACTIVATE per k-tile PAIR ([128,1024]
      PSUM span) -- amortizes the ACT 352-cycle fixed cost
    colsum[q] += ones^T ET ; OT[dh,q] += V_h[ktile]^T ET  (bf16 moving)
    OTs = OT * broadcast(1/colsum)
  phase 4: y_partial[q,:] = sum_h OTs_h[:,qtile]^T Wo_sl_h (PSUM->SBUF
    copies alternate DVE/ACT; yp tiles share the p3ot PSUM pool)
Host: y[b] = sum of 4 partials + bo.

Matmuls f32r (1 cycle/row) except attention-prob matmuls (bf16 x bf16).
Rule discovered empirically: the MOVING operand of an f32r matmul must be
produced by instructions writing f32r-typed APs; the stationary operand can
be f32-written and bitcast at the matmul.
"""
import sys

sys.path.insert(0, "/opt/trn_rl_repo")

import math
import numpy as np

import concourse.bass as bass
import concourse.mybir as mybir
import concourse.tile as tile
from concourse import bacc
from concourse.bass_utils import run_bass_kernel_spmd

F32 = mybir.dt.float32
F32R = mybir.dt.float32r
BF16 = mybir.dt.bfloat16
EXP = mybir.ActivationFunctionType.Exp
IDENT = mybir.ActivationFunctionType.Identity
MULT = mybir.AluOpType.mult
ADD = mybir.AluOpType.add

D_MODEL = 2048
NUM_HEADS = 16
D_HEAD = 128
D_LATENT = 512
B, S = 2, 2048
N_CORES = 8
HG = 4                      # head-groups (cores per batch)
HPC = NUM_HEADS // HG       # heads per core = 4
HSL = HPC * D_HEAD          # head-dim slice per core = 512
NQT = S // 128              # 16 q tiles of 128
NJ = S // 512               # 4 q chunks of 512
NKK = D_MODEL // 128        # 16 contraction chunks
NLK = D_LATENT // 128       # 4 latent chunks
SCALE = 1.0 / math.sqrt(D_HEAD)
NEG = -1.0e9

_BUILD_CACHE = {}


def build(causal: bool):
    if causal in _BUILD_CACHE:
        return _BUILD_CACHE[causal]
    nc = bacc.Bacc("TRN2", target_bir_lowering=False, debug=False,
                   num_devices=N_CORES)

    xT = nc.dram_tensor("xT", [D_MODEL, S], F32, kind="ExternalInput").ap()
    wq = nc.dram_tensor("wq", [D_MODEL, HSL], F32, kind="ExternalInput").ap()
    bq = nc.dram_tensor("bq", [HPC, 128], F32, kind="ExternalInput").ap()
    wkv = nc.dram_tensor("wkv", [D_MODEL, D_LATENT], F32, kind="ExternalInput").ap()
    bkv = nc.dram_tensor("bkv", [NLK, 128], F32, kind="ExternalInput").ap()
    wk = nc.dram_tensor("wk", [D_LATENT, HSL], F32, kind="ExternalInput").ap()
    bk = nc.dram_tensor("bk", [HPC, 128], F32, kind="ExternalInput").ap()
    wv = nc.dram_tensor("wv", [D_LATENT, HSL], F32, kind="ExternalInput").ap()
    bv = nc.dram_tensor("bv", [1, HSL], F32, kind="ExternalInput").ap()
    wo = nc.dram_tensor("wo", [HSL, D_MODEL], F32, kind="ExternalInput").ap()
    if causal:
        maskb = nc.dram_tensor("maskb", [128, 128], F32,
                               kind="ExternalInput").ap()
    else:
        maskb = nc.dram_tensor("maskb", [S, S], F32, kind="ExternalInput").ap()
    y = nc.dram_tensor("y", [S, D_MODEL], F32, kind="ExternalOutput").ap()

    xTr = xT.rearrange("(kk p) s -> kk p s", p=128)

    with tile.TileContext(nc) as tc:
        from contextlib import ExitStack
        with ExitStack() as ctx:
            # ---- pools, strictly LIFO lifetimes ----
            persist = ctx.enter_context(tc.tile_pool(name="persist", bufs=1))
            qt_sb = persist.tile([128, HPC, S], F32R, tag="qt")
            consts = ctx.enter_context(tc.tile_pool(name="consts", bufs=1))
            kv_pool = ctx.enter_context(tc.tile_pool(name="kvp", bufs=1))
            kt_sb = kv_pool.tile([128, HPC, S], F32R, tag="kt")
            v_sb = kv_pool.tile([128, NQT, HSL], BF16, tag="v")
            # wk/wv/bvb live beside phase 1 so their DMAs stream during it
            pw2 = ctx.enter_context(tc.tile_pool(name="pw2", bufs=1))
            wk_sb = pw2.tile([128, NLK, HSL], F32R, tag="wk")
            wv_sb = pw2.tile([128, NLK, HSL], F32R, tag="wv")
            bvb_sb = pw2.tile([128, HSL], F32, tag="bvb")
            es_lat = ExitStack()   # lat: phases 1-2
            lat_pool = es_lat.enter_context(tc.tile_pool(name="latp", bufs=1))
            lat_sb = lat_pool.tile([128, NLK, S], F32R, tag="lat")
            ones1 = consts.tile([1, 128], F32, tag="ones1")
            onesk_bf = consts.tile([128, 1], BF16, tag="oneskb")
            bq_sb = consts.tile([128, HPC], F32, tag="bq")
            bkv_sb = consts.tile([128, NLK], F32, tag="bkv")
            bk_sb = consts.tile([128, HPC], F32, tag="bk")
            bv_sb = consts.tile([1, HSL], F32R, tag="bv")
            if causal:
                mb_sb = consts.tile([128, 128], F32, tag="mb")

            nc.vector.memset(ones1, 1.0)
            nc.vector.memset(onesk_bf, 1.0)

            # ---- phase 1 (fused): latT = Wkv^T xT + bkv ; QT = Wq^T xT + bq
            with tc.tile_pool(name="pW", bufs=1) as pW, \
                 tc.tile_pool(name="pAx", bufs=4) as pAx, \
                 tc.tile_pool(name="ph1", bufs=4, space="PSUM") as ph1:
                wkv_sb = pW.tile([128, NKK, D_LATENT], F32R, tag="wkv")
                wq_sb = pW.tile([128, NKK, HSL], F32R, tag="wq")
                wkvr = wkv.rearrange("(kk p) m -> kk p m", p=128).bitcast(F32R)
                wqr = wq.rearrange("(kk p) m -> kk p m", p=128).bitcast(F32R)
                for kk in range(NKK):
                    nc.scalar.dma_start(out=wkv_sb[:, kk, :], in_=wkvr[kk])
                    nc.scalar.dma_start(out=wq_sb[:, kk, :], in_=wqr[kk])
                nc.scalar.dma_start(out=bkv_sb, in_=bkv.rearrange("m p -> p m"))
                nc.scalar.dma_start(out=bq_sb, in_=bq.rearrange("m p -> p m"))
                nc.scalar.dma_start(out=bk_sb, in_=bk.rearrange("m p -> p m"))
                nc.scalar.dma_start(out=bv_sb, in_=bv.bitcast(F32R))
                if causal:
                    nc.scalar.dma_start(out=mb_sb, in_=maskb)
                nc.gpsimd.dma_start(out=wk_sb,
                                    in_=wk.rearrange("(lk p) m -> p lk m", p=128).bitcast(F32R))
                nc.gpsimd.dma_start(out=wv_sb,
                                    in_=wv.rearrange("(lk p) m -> p lk m", p=128).bitcast(F32R))
                for sc in range(NJ):
                    lps = [ph1.tile([128, 512], F32, tag="lp", name=f"lp{m}")
                           for m in range(NLK)]
                    qps = [ph1.tile([128, 512], F32, tag="qp", name=f"qp{m}")
                           for m in range(HPC)]
                    for kk in range(NKK):
                        xt = pAx.tile([128, 512], F32R, tag="xt")
                        nc.sync.dma_start(
                            out=xt,
                            in_=xTr[kk, :, sc * 512:(sc + 1) * 512].bitcast(F32R))
                        for m in range(NLK):
                            nc.tensor.matmul(
                                lps[m][:],
                                wkv_sb[:, kk, m * 128:(m + 1) * 128],
                                xt[:],
                                start=(kk == 0), stop=(kk == NKK - 1))
                        for m in range(HPC):
                            nc.tensor.matmul(
                                qps[m][:],
                                wq_sb[:, kk, m * 128:(m + 1) * 128],
                                xt[:],
                                start=(kk == 0), stop=(kk == NKK - 1))
                    for m in range(NLK):
                        nc.scalar.activation(
                            lat_sb[:, m, sc * 512:(sc + 1) * 512], lps[m][:],
                            IDENT, bias=bkv_sb[:, m:m + 1])
                    for m in range(HPC):
                        nc.vector.tensor_scalar_add(
                            qt_sb[:, m, sc * 512:(sc + 1) * 512], qps[m][:],
                            bq_sb[:, m:m + 1])

            # ---- phase 2: KT + V from latT ----
            with tc.tile_pool(name="ph2k", bufs=4, space="PSUM") as ph2k, \
                 tc.tile_pool(name="ph2v", bufs=4, space="PSUM") as ph2v:
                # broadcast bv to [128, HSL] once (for the fused V readout)
                bvp = ph2v.tile([128, HSL], F32, tag="vp", name="bvp")
                nc.tensor.matmul(bvp[:], ones1[0:1, :].bitcast(F32R),
                                 bv_sb[0:1, :], start=True, stop=True)
                nc.vector.tensor_copy(bvb_sb, bvp[:])
                for sc in range(NJ):
                    for dm in range(HPC):
                        kp = ph2k.tile([128, 512], F32, tag="kp")
                        for lk in range(NLK):
                            nc.tensor.matmul(
                                kp[:],
                                wk_sb[:, lk, dm * 128:(dm + 1) * 128],
                                lat_sb[:, lk, sc * 512:(sc + 1) * 512],
                                start=(lk == 0), stop=(lk == NLK - 1))
                        nc.scalar.activation(
                            kt_sb[:, dm, sc * 512:(sc + 1) * 512], kp[:],
                            IDENT, bias=bk_sb[:, dm:dm + 1])
                    for ti in range(4):
                        t = sc * 4 + ti
                        vp = ph2v.tile([128, 512], F32, tag="vp")
                        for lk in range(NLK):
                            nc.tensor.matmul(
                                vp[:],
                                lat_sb[:, lk, t * 128:(t + 1) * 128],
                                wv_sb[:, lk, :],
                                start=(lk == 0), stop=(lk == NLK - 1))
                        nc.vector.scalar_tensor_tensor(
                            v_sb[:, t, :], vp[:], 1.0, bvb_sb, MULT, ADD)

            es_lat.close()  # free latT space

            # ---- phases 3+4: attention + output projection, per q-chunk ----
            p34 = ctx.enter_context(tc.tile_pool(name="p34", bufs=1))
            ots_sb = p34.tile([128, HPC, S], F32R, tag="ots")
            wo_sb = p34.tile([128, HPC, D_MODEL], F32R, tag="wo")
            wor = wo.rearrange("(h p) m -> h p m", p=128).bitcast(F32R)
            for h_ in range(HPC):
                nc.scalar.dma_start(out=wo_sb[:, h_, :], in_=wor[h_])
            with tc.tile_pool(name="p3st", bufs=1, space="PSUM") as p3st, \
                 tc.tile_pool(name="p3cs", bufs=1, space="PSUM") as p3cs, \
                 tc.tile_pool(name="p3ot", bufs=2, space="PSUM") as p3ot, \
                 tc.tile_pool(name="p3rb", bufs=1, space="PSUM") as p3rb, \
                 tc.tile_pool(name="p3et", bufs=4) as p3et, \
                 tc.tile_pool(name="p3sb", bufs=4) as p3sb, \
                 tc.tile_pool(name="p4sb", bufs=3) as p4sb:
                for j in range(NJ):
                    n_i2 = 4 * j + 4 if causal else NQT
                    for h in range(HPC):
                        ot = p3ot.tile([128, 512], F32, tag="ot")
                        cs = p3cs.tile([1, 512], F32, tag="cs")

                        def emit_csot(halves, et):
                            for sl, i2 in halves:
                                r = i2 - 4 * j
                                cr = 128 * r if (causal and r >= 0) else 0
                                nc.tensor.matmul(
                                    cs[0:1, cr:512], onesk_bf[:, 0:1],
                                    et[:, sl * 512 + cr:(sl + 1) * 512],
                                    start=(i2 == 0), stop=(i2 == n_i2 - 1))
                            for sl, i2 in halves:
                                r = i2 - 4 * j
                                cr = 128 * r if (causal and r >= 0) else 0
                                nc.tensor.matmul(
                                    ot[:, cr:512],
                                    v_sb[:, i2, h * 128:(h + 1) * 128],
                                    et[:, sl * 512 + cr:(sl + 1) * 512],
                                    start=(i2 == 0), stop=(i2 == n_i2 - 1))

                        pend = []  # software pipeline: cs/ot lag one group
                        for pi in range(n_i2 // 4):
                            halves = [(sl, 4 * pi + sl) for sl in range(4)]
                            st = p3st.tile([128, 2048], F32, tag="st")
                            for sl, i2 in halves:
                                r = i2 - 4 * j
                                cr = 128 * r if (causal and r >= 0) else 0
                                nc.tensor.matmul(
                                    st[:, sl * 512:(sl + 1) * 512],
                                    kt_sb[:, h, i2 * 128:(i2 + 1) * 128],
                                    qt_sb[:, h, j * 512:(j + 1) * 512],
                                    start=True, stop=True)
                                if causal and r >= 0:
                                    o = sl * 512 + cr
                                    nc.vector.tensor_add(
                                        st[:, o:o + 128], st[:, o:o + 128],
                                        mb_sb)
                                elif not causal:
                                    mt = p3sb.tile([128, 512], F32, tag="mt")
                                    nc.sync.dma_start(
                                        out=mt,
                                        in_=maskb.rearrange(
                                            "(i p) q -> i p q", p=128)
                                        [i2, :, j * 512:(j + 1) * 512])
                                    nc.vector.tensor_add(
                                        st[:, sl * 512:(sl + 1) * 512],
                                        st[:, sl * 512:(sl + 1) * 512], mt[:])
                            et = p3et.tile([128, 2048], BF16, tag="et")
                            nc.scalar.activation(et[:], st[:], EXP, scale=SCALE)
                            pend.append((halves, et))
                            if len(pend) > 1:
                                emit_csot(*pend.pop(0))
                        for pe_ in pend:
                            emit_csot(*pe_)
                        pend = []
                        csb = p3sb.tile([1, 512], F32R, tag="csb")
                        nc.vector.tensor_copy(csb[0:1, :], cs[0:1, :])
                        rb = p3rb.tile([128, 512], F32, tag="rb")
                        nc.tensor.matmul(rb[:], ones1[0:1, :].bitcast(F32R),
                                         csb[0:1, :], start=True, stop=True)
                        rs = p3sb.tile([128, 512], F32, tag="rs")
                        nc.vector.reciprocal_approx_fast(out=rs[:], in_=rb[:])
                        nc.vector.tensor_mul(
                            ots_sb[:, h, j * 512:(j + 1) * 512], ot[:], rs[:])
                    # phase 4 for this q chunk (yp shares the p3ot pool)
                    for ti in range(4):
                        t = 4 * j + ti
                        for yc in range(NJ):
                            yp = p3ot.tile([128, 512], F32, tag="ot",
                                           name="yp")
                            for h in range(HPC):
                                nc.tensor.matmul(
                                    yp[:],
                                    ots_sb[:, h, t * 128:(t + 1) * 128],
                                    wo_sb[:, h, yc * 512:(yc + 1) * 512],
                                    start=(h == 0), stop=(h == HPC - 1))
                            ys = p4sb.tile([128, 512], F32, tag="ys")
                            nc.vector.tensor_copy(ys[:], yp[:])
                            nc.sync.dma_start(
                                out=y[t * 128:(t + 1) * 128,
                                      yc * 512:(yc + 1) * 512],
                                in_=ys[:])

    nc.compile()
    _BUILD_CACHE[causal] = nc
    return nc


def kernel(**inputs) -> np.ndarray:
    x = np.asarray(inputs["x"], dtype=np.float32)
    mask = np.asarray(inputs["mask"])
    Wq = np.asarray(inputs["Wq"], dtype=np.float32)
    bq = np.asarray(inputs["bq"], dtype=np.float32)
    Wkv = np.asarray(inputs["Wkv"], dtype=np.float32)
    bkv = np.asarray(inputs["bkv"], dtype=np.float32)
    Wk = np.asarray(inputs["Wk"], dtype=np.float32)
    bk = np.asarray(inputs["bk"], dtype=np.float32)
    Wv = np.asarray(inputs["Wv"], dtype=np.float32)
    bv = np.asarray(inputs["bv"], dtype=np.float32)
    Wo = np.asarray(inputs["Wo"], dtype=np.float32)
    bo = np.asarray(inputs["bo"], dtype=np.float32)

    tril = np.tril(np.ones((S, S), dtype=mask.dtype))
    causal = all(np.array_equal(mask[b], tril) for b in range(B))
    nc = build(causal)

    # canonical causal diagonal-subtile additive bias [p, f]: NEG where f < p
    if causal:
        p = np.arange(128)[:, None]
        f = np.arange(128)[None, :]
        mb = np.where(f < p, NEG, 0.0).astype(np.float32)

    in_maps = []
    for c in range(N_CORES):
        b, g = divmod(c, HG)
        sl = slice(g * HSL, (g + 1) * HSL)
        m = {
            "xT": np.ascontiguousarray(x[b].T),
            "wq": np.ascontiguousarray(Wq[:, sl]),
            "bq": np.ascontiguousarray(bq[sl]).reshape(HPC, 128),
            "wkv": Wkv,
            "bkv": bkv.reshape(NLK, 128),
            "wk": np.ascontiguousarray(Wk[:, sl]),
            "bk": np.ascontiguousarray(bk[sl]).reshape(HPC, 128),
            "wv": np.ascontiguousarray(Wv[:, sl]),
            "bv": np.ascontiguousarray(bv[sl]).reshape(1, HSL),
            "wo": np.ascontiguousarray(Wo[sl, :]),
        }
        if causal:
            m["maskb"] = mb
        else:
            m["maskb"] = np.ascontiguousarray(
                np.where(mask[b] == 0, NEG, 0.0).astype(np.float32).T)
        in_maps.append(m)

    res = run_bass_kernel_spmd(nc, in_maps, list(range(N_CORES)))
    out = np.empty((B, S, D_MODEL), dtype=np.float32)
    for b in range(B):
        acc = res.results[b * HG]["y"].astype(np.float32).copy()
        for g in range(1, HG):
            acc += res.results[b * HG + g]["y"]
        out[b] = acc + bo
    return out


# revision 17
# speedup vs baseline: 1.1112x; 1.1112x over previous
"""MultiHeadLatentAttention Trainium2 kernel (8 NeuronCores, SPMD).

Sharding: batch (2) x head-group (4 of 4 heads each) -> 8 cores.
Each core computes, for its batch b and heads [4g, 4g+4):
  fused phase 1: latT = Wkv^T x^T + bkv AND QT = Wq^T x^T + bq from one
    xT stream (xT DMA'd once, 8 PSUM banks: 4 lat + 4 q per seq chunk;
    lat readout on ACT with fused bias, q readout on DVE)
  phase 2: KT = Wk_sl^T latT + bk (ACT readout+bias);
    V = latT^T Wv_sl + bv (DVE fused copy+bias, written bf16)
  phase 3 per head h, q-chunk j (512 wide), causal k-tiles paired:
    ST[k,q] = KT_h[:,ktile]^T QT_h[:,j]   (diagonal tiles: only cols
      >= 128*r computed; [128,128] triangle bias added on the diagonal)
    ET = exp(scale*ST) via ONE ACTIVATE per k-tile PAIR ([128,1024]
      PSUM span) -- amortizes the ACT 352-cycle fixed cost
    colsum[q] += ones^T ET ; OT[dh,q] += V_h[ktile]^T ET  (bf16 moving)
    OTs = OT * broadcast(1/colsum)
  phase 4: y_partial[q,:] = sum_h OTs_h[:,qtile]^T Wo_sl_h (PSUM->SBUF
    copies alternate DVE/ACT; yp tiles share the p3ot PSUM pool)
Host: y[b] = sum of 4 partials + bo.

Matmuls f32r (1 cycle/row) except attention-prob matmuls (bf16 x bf16).
Rule discovered empirically: the MOVING operand of an f32r matmul must be
produced by instructions writing f32r-typed APs; the stationary operand can
be f32-written and bitcast at the matmul.
"""
import sys

sys.path.insert(0, "/opt/trn_rl_repo")

import math
import numpy as np

import concourse.bass as bass
import concourse.mybir as mybir
import concourse.tile as tile
from concourse import bacc
from concourse.bass_utils import run_bass_kernel_spmd

F32 = mybir.dt.float32
F32R = mybir.dt.float32r
BF16 = mybir.dt.bfloat16
EXP = mybir.ActivationFunctionType.Exp
IDENT = mybir.ActivationFunctionType.Identity
MULT = mybir.AluOpType.mult
ADD = mybir.AluOpType.add

D_MODEL = 2048
NUM_HEADS = 16
D_HEAD = 128
D_LATENT = 512
B, S = 2, 2048
N_CORES = 8
HG = 4                      # head-groups (cores per batch)
HPC = NUM_HEADS // HG       # heads per core = 4
HSL = HPC * D_HEAD          # head-dim slice per core = 512
NQT = S // 128              # 16 q tiles of 128
NJ = S // 512               # 4 q chunks of 512
NKK = D_MODEL // 128        # 16 contraction chunks
NLK = D_LATENT // 128       # 4 latent chunks
SCALE = 1.0 / math.sqrt(D_HEAD)
NEG = -1.0e9

_BUILD_CACHE = {}


def build(causal: bool):
    if causal in _BUILD_CACHE:
        return _BUILD_CACHE[causal]
    nc = bacc.Bacc("TRN2", target_bir_lowering=False, debug=False,
                   num_devices=N_CORES)

    xT = nc.dram_tensor("xT", [D_MODEL, S], F32, kind="ExternalInput").ap()
    wq = nc.dram_tensor("wq", [D_MODEL, HSL], F32, kind="ExternalInput").ap()
    bq = nc.dram_tensor("bq", [HPC, 128], F32, kind="ExternalInput").ap()
    wkv = nc.dram_tensor("wkv", [D_MODEL, D_LATENT], F32, kind="ExternalInput").ap()
    bkv = nc.dram_tensor("bkv", [NLK, 128], F32, kind="ExternalInput").ap()
    wk = nc.dram_tensor("wk", [D_LATENT, HSL], F32, kind="ExternalInput").ap()
    bk = nc.dram_tensor("bk", [HPC, 128], F32, kind="ExternalInput").ap()
    wv = nc.dram_tensor("wv", [D_LATENT, HSL], F32, kind="ExternalInput").ap()
    bv = nc.dram_tensor("bv", [1, HSL], F32, kind="ExternalInput").ap()
    wo = nc.dram_tensor("wo", [HSL, D_MODEL], F32, kind="ExternalInput").ap()
    if causal:
        maskb = nc.dram_tensor("maskb", [128, 128], F32,
                               kind="ExternalInput").ap()
    else:
        maskb = nc.dram_tensor("maskb", [S, S], F32, kind="ExternalInput").ap()
    y = nc.dram_tensor("y", [S, D_MODEL], F32, kind="ExternalOutput").ap()

    xTr = xT.rearrange("(kk p) s -> kk p s", p=128)

    with tile.TileContext(nc) as tc:
        from contextlib import ExitStack
        with ExitStack() as ctx:
            # ---- pools, strictly LIFO lifetimes ----
            persist = ctx.enter_context(tc.tile_pool(name="persist", bufs=1))
            qt_sb = persist.tile([128, HPC, S], F32R, tag="qt")
            consts = ctx.enter_context(tc.tile_pool(name="consts", bufs=1))
            kv_pool = ctx.enter_context(tc.tile_pool(name="kvp", bufs=1))
            kt_sb = kv_pool.tile([128, HPC, S], F32R, tag="kt")
            v_sb = kv_pool.tile([128, NQT, HSL], BF16, tag="v")
            # wk/wv/bvb live beside phase 1 so their DMAs stream during it
            pw2 = ctx.enter_context(tc.tile_pool(name="pw2", bufs=1))
            wk_sb = pw2.tile([128, NLK, HSL], F32R, tag="wk")
            wv_sb = pw2.tile([128, NLK, HSL], F32R, tag="wv")
            bvb_sb = pw2.tile([128, HSL], F32, tag="bvb")
            es_lat = ExitStack()   # lat: phases 1-2
            lat_pool = es_lat.enter_context(tc.tile_pool(name="latp", bufs=1))
            lat_sb = lat_pool.tile([128, NLK, S], F32R, tag="lat")
            ones1 = consts.tile([1, 128], F32, tag="ones1")
            onesk_bf = consts.tile([128, 1], BF16, tag="oneskb")
            bq_sb = consts.tile([128, HPC], F32, tag="bq")
            bkv_sb = consts.tile([128, NLK], F32, tag="bkv")
            bk_sb = consts.tile([128, HPC], F32, tag="bk")
            bv_sb = consts.tile([1, HSL], F32R, tag="bv")
            if causal:
                mb_sb = consts.tile([128, 128], F32, tag="mb")

            nc.vector.memset(ones1, 1.0)
            nc.vector.memset(onesk_bf, 1.0)

            # ---- phase 1 (fused): latT = Wkv^T xT + bkv ; QT = Wq^T xT + bq
            with tc.tile_pool(name="pW", bufs=1) as pW, \
                 tc.tile_pool(name="pAx", bufs=4) as pAx, \
                 tc.tile_pool(name="ph1", bufs=4, space="PSUM") as ph1:
                wkv_sb = pW.tile([128, NKK, D_LATENT], F32R, tag="wkv")
                wq_sb = pW.tile([128, NKK, HSL], F32R, tag="wq")
                wkvr = wkv.rearrange("(kk p) m -> kk p m", p=128).bitcast(F32R)
                wqr = wq.rearrange("(kk p) m -> kk p m", p=128).bitcast(F32R)
                for kk in range(NKK):
                    nc.scalar.dma_start(out=wkv_sb[:, kk, :], in_=wkvr[kk])
                    nc.scalar.dma_start(out=wq_sb[:, kk, :], in_=wqr[kk])
                nc.scalar.dma_start(out=bkv_sb, in_=bkv.rearrange("m p -> p m"))
                nc.scalar.dma_start(out=bq_sb, in_=bq.rearrange("m p -> p m"))
                nc.scalar.dma_start(out=bk_sb, in_=bk.rearrange("m p -> p m"))
                nc.scalar.dma_start(out=bv_sb, in_=bv.bitcast(F32R))
                if causal:
                    nc.scalar.dma_start(out=mb_sb, in_=maskb)
                nc.gpsimd.dma_start(out=wk_sb,
                                    in_=wk.rearrange("(lk p) m -> p lk m", p=128).bitcast(F32R))
                nc.gpsimd.dma_start(out=wv_sb,
                                    in_=wv.rearrange("(lk p) m -> p lk m", p=128).bitcast(F32R))
                for sc in range(NJ):
                    lps = [ph1.tile([128, 512], F32, tag="lp", name=f"lp{m}")
                           for m in range(NLK)]
                    qps = [ph1.tile([128, 512], F32, tag="qp", name=f"qp{m}")
                           for m in range(HPC)]
                    for kk in range(NKK):
                        xt = pAx.tile([128, 512], F32R, tag="xt")
                        nc.sync.dma_start(
                            out=xt,
                            in_=xTr[kk, :, sc * 512:(sc + 1) * 512].bitcast(F32R))
                        for m in range(NLK):
                            nc.tensor.matmul(
                                lps[m][:],
                                wkv_sb[:, kk, m * 128:(m + 1) * 128],
                                xt[:],
                                start=(kk == 0), stop=(kk == NKK - 1))
                        for m in range(HPC):
                            nc.tensor.matmul(
                                qps[m][:],
                                wq_sb[:, kk, m * 128:(m + 1) * 128],
                                xt[:],
                                start=(kk == 0), stop=(kk == NKK - 1))
                    for m in range(NLK):
                        nc.scalar.activation(
                            lat_sb[:, m, sc * 512:(sc + 1) * 512], lps[m][:],
                            IDENT, bias=bkv_sb[:, m:m + 1])
                    for m in range(HPC):
                        nc.vector.tensor_scalar_add(
                            qt_sb[:, m, sc * 512:(sc + 1) * 512], qps[m][:],
                            bq_sb[:, m:m + 1])

            # ---- phase 2: KT + V from latT ----
            with tc.tile_pool(name="ph2k", bufs=4, space="PSUM") as ph2k, \
                 tc.tile_pool(name="ph2v", bufs=4, space="PSUM") as ph2v:
                # broadcast bv to [128, HSL] once (for the fused V readout)
                bvp = ph2v.tile([128, HSL], F32, tag="vp", name="bvp")
                nc.tensor.matmul(bvp[:], ones1[0:1, :].bitcast(F32R),
                                 bv_sb[0:1, :], start=True, stop=True)
                nc.vector.tensor_copy(bvb_sb, bvp[:])
                for sc in range(NJ):
                    for dm in range(HPC):
                        kp = ph2k.tile([128, 512], F32, tag="kp")
                        for lk in range(NLK):
                            nc.tensor.matmul(
                                kp[:],
                                wk_sb[:, lk, dm * 128:(dm + 1) * 128],
                                lat_sb[:, lk, sc * 512:(sc + 1) * 512],
                                start=(lk == 0), stop=(lk == NLK - 1))
                        nc.scalar.activation(
                            kt_sb[:, dm, sc * 512:(sc + 1) * 512], kp[:],
                            IDENT, bias=bk_sb[:, dm:dm + 1])
                    for ti in range(4):
                        t = sc * 4 + ti
                        vp = ph2v.tile([128, 512], F32, tag="vp")
                        for lk in range(NLK):
                            nc.tensor.matmul(
                                vp[:],
                                lat_sb[:, lk, t * 128:(t + 1) * 128],
                                wv_sb[:, lk, :],
                                start=(lk == 0), stop=(lk == NLK - 1))
                        nc.vector.scalar_tensor_tensor(
                            v_sb[:, t, :], vp[:], 1.0, bvb_sb, MULT, ADD)

            es_lat.close()  # free latT space

            # ---- phases 3+4: attention + output projection, per q-chunk ----
            p34 = ctx.enter_context(tc.tile_pool(name="p34", bufs=1))
            ots_sb = p34.tile([128, HPC, S], F32R, tag="ots")
            wo_sb = p34.tile([128, HPC, D_MODEL], F32R, tag="wo")
            wor = wo.rearrange("(h p) m -> h p m", p=128).bitcast(F32R)
            for h_ in range(HPC):
                nc.scalar.dma_start(out=wo_sb[:, h_, :], in_=wor[h_])
            with tc.tile_pool(name="p3st", bufs=2, space="PSUM") as p3st, \
                 tc.tile_pool(name="p3cs", bufs=1, space="PSUM") as p3cs, \
                 tc.tile_pool(name="p3ot", bufs=2, space="PSUM") as p3ot, \
                 tc.tile_pool(name="p3rb", bufs=1, space="PSUM") as p3rb, \
                 tc.tile_pool(name="p3et", bufs=4) as p3et, \
                 tc.tile_pool(name="p3sb", bufs=4) as p3sb:
                for j in range(NJ):
                    n_i2 = 4 * j + 4 if causal else NQT
                    for h in range(HPC):
                        ot = p3ot.tile([128, 512], F32, tag="ot")
                        cs = p3cs.tile([1, 512], F32, tag="cs")

                        def emit_csot(halves, et):
                            for sl, i2 in halves:
                                r = i2 - 4 * j
                                cr = 128 * r if (causal and r >= 0) else 0
                                nc.tensor.matmul(
                                    cs[0:1, cr:512], onesk_bf[:, 0:1],
                                    et[:, sl * 512 + cr:(sl + 1) * 512],
                                    start=(i2 == 0), stop=(i2 == n_i2 - 1))
                            for sl, i2 in halves:
                                r = i2 - 4 * j
                                cr = 128 * r if (causal and r >= 0) else 0
                                nc.tensor.matmul(
                                    ot[:, cr:512],
                                    v_sb[:, i2, h * 128:(h + 1) * 128],
                                    et[:, sl * 512 + cr:(sl + 1) * 512],
                                    start=(i2 == 0), stop=(i2 == n_i2 - 1))

                        pend = []  # software pipeline: cs/ot lag two pairs
                        for pi in range(n_i2 // 2):
                            halves = [(sl, 2 * pi + sl) for sl in range(2)]
                            st = p3st.tile([128, 1024], F32, tag="st")
                            for sl, i2 in halves:
                                r = i2 - 4 * j
                                cr = 128 * r if (causal and r >= 0) else 0
                                nc.tensor.matmul(
                                    st[:, sl * 512:(sl + 1) * 512],
                                    kt_sb[:, h, i2 * 128:(i2 + 1) * 128],
                                    qt_sb[:, h, j * 512:(j + 1) * 512],
                                    start=True, stop=True)
                                if causal and r >= 0:
                                    o = sl * 512 + cr
                                    nc.vector.tensor_add(
                                        st[:, o:o + 128], st[:, o:o + 128],
                                        mb_sb)
                                elif not causal:
                                    mt = p3sb.tile([128, 512], F32, tag="mt")
                                    nc.sync.dma_start(
                                        out=mt,
                                        in_=maskb.rearrange(
                                            "(i p) q -> i p q", p=128)
                                        [i2, :, j * 512:(j + 1) * 512])
                                    nc.vector.tensor_add(
                                        st[:, sl * 512:(sl + 1) * 512],
                                        st[:, sl * 512:(sl + 1) * 512], mt[:])
                            et = p3et.tile([128, 1024], BF16, tag="et")
                            nc.scalar.activation(et[:], st[:], EXP, scale=SCALE)
                            pend.append((halves, et))
                            if len(pend) > 2:
                                emit_csot(*pend.pop(0))
                        for pe_ in pend:
                            emit_csot(*pe_)
                        pend = []
                        csb = p3sb.tile([1, 512], F32R, tag="csb")
                        nc.vector.tensor_copy(csb[0:1, :], cs[0:1, :])
                        rb = p3rb.tile([128, 512], F32, tag="rb")
                        nc.tensor.matmul(rb[:], ones1[0:1, :].bitcast(F32R),
                                         csb[0:1, :], start=True, stop=True)
                        rs = p3sb.tile([128, 512], F32, tag="rs")
                        nc.vector.reciprocal_approx_fast(out=rs[:], in_=rb[:])
                        nc.vector.tensor_mul(
                            ots_sb[:, h, j * 512:(j + 1) * 512], ot[:], rs[:])
            # ---- phase 4: output projection, all q tiles (dense PE) ----
            with tc.tile_pool(name="p4y", bufs=4, space="PSUM") as p4y, \
                 tc.tile_pool(name="p4sb", bufs=4) as p4sb:
                for t in range(NQT):
                    for yc in range(NJ):
                        yp = p4y.tile([128, 512], F32, tag="yp")
                        for h in range(HPC):
                            nc.tensor.matmul(
                                yp[:],
                                ots_sb[:, h, t * 128:(t + 1) * 128],
                                wo_sb[:, h, yc * 512:(yc + 1) * 512],
                                start=(h == 0), stop=(h == HPC - 1))
                        ys = p4sb.tile([128, 512], F32, tag="ys")
                        if (t * NJ + yc) % 2 == 0:
                            nc.vector.tensor_copy(ys[:], yp[:])
                        else:
                            nc.scalar.copy(ys[:], yp[:])
                        nc.sync.dma_start(
                            out=y[t * 128:(t + 1) * 128,
                                  yc * 512:(yc + 1) * 512],
                            in_=ys[:])

    nc.compile()
    _BUILD_CACHE[causal] = nc
    return nc


def kernel(**inputs) -> np.ndarray:
    x = np.asarray(inputs["x"], dtype=np.float32)
    mask = np.asarray(inputs["mask"])
    Wq = np.asarray(inputs["Wq"], dtype=np.float32)
    bq = np.asarray(inputs["bq"], dtype=np.float32)
    Wkv = np.asarray(inputs["Wkv"], dtype=np.float32)
    bkv = np.asarray(inputs["bkv"], dtype=np.float32)
    Wk = np.asarray(inputs["Wk"], dtype=np.float32)
    bk = np.asarray(inputs["bk"], dtype=np.float32)
    Wv = np.asarray(inputs["Wv"], dtype=np.float32)
    bv = np.asarray(inputs["bv"], dtype=np.float32)
    Wo = np.asarray(inputs["Wo"], dtype=np.float32)
    bo = np.asarray(inputs["bo"], dtype=np.float32)

    tril = np.tril(np.ones((S, S), dtype=mask.dtype))
    causal = all(np.array_equal(mask[b], tril) for b in range(B))
    nc = build(causal)

    # canonical causal diagonal-subtile additive bias [p, f]: NEG where f < p
    if causal:
        p = np.arange(128)[:, None]
        f = np.arange(128)[None, :]
        mb = np.where(f < p, NEG, 0.0).astype(np.float32)

    in_maps = []
    for c in range(N_CORES):
        b, g = divmod(c, HG)
        sl = slice(g * HSL, (g + 1) * HSL)
        m = {
            "xT": np.ascontiguousarray(x[b].T),
            "wq": np.ascontiguousarray(Wq[:, sl]),
            "bq": np.ascontiguousarray(bq[sl]).reshape(HPC, 128),
            "wkv": Wkv,
            "bkv": bkv.reshape(NLK, 128),
            "wk": np.ascontiguousarray(Wk[:, sl]),
            "bk": np.ascontiguousarray(bk[sl]).reshape(HPC, 128),
            "wv": np.ascontiguousarray(Wv[:, sl]),
            "bv": np.ascontiguousarray(bv[sl]).reshape(1, HSL),
            "wo": np.ascontiguousarray(Wo[sl, :]),
        }
        if causal:
            m["maskb"] = mb
        else:
            m["maskb"] = np.ascontiguousarray(
                np.where(mask[b] == 0, NEG, 0.0).astype(np.float32).T)
        in_maps.append(m)

    res = run_bass_kernel_spmd(nc, in_maps, list(range(N_CORES)))
    out = np.empty((B, S, D_MODEL), dtype=np.float32)
    for b in range(B):
        acc = res.results[b * HG]["y"].astype(np.float32).copy()
        for g in range(1, HG):
            acc += res.results[b * HG + g]["y"]
        out[b] = acc + bo
    return out
